# revision 24
# baseline (speedup 1.0000x reference)
"""Trainium2 Bass kernel for nn_ContextGatingSigmoidClassifier.

Math (eval mode):
  f_so = lrelu(W_so @ bn_so(x_so) + b_so)        x: [B,2048,N,H,W]
  f_c  = lrelu(W_c  @ bn_c(x_c)  + b_c)
  f    = concat -> bn1 -> W1 -> bn2 -> lrelu -> W2 -> mean(H,W) -> sigmoid > 0.5

All BatchNorms are eval-mode affine maps, so they fold into the adjacent
linear layers (done host-side in fp64). Final threshold:
  sigmoid(mean) > 0.5  <=>  sum_hw(W2 @ h) > -49*b2.

Device mapping: data-parallel over batch (4 per core, 8 cores), weights
replicated, x cast fp32->fp16 host-side. The kernel is tensor-engine
bound (1152 matmuls of N=294 at 125ns = 144us); the schedule exists to
keep the PE stream dense from the first possible cycle:
  - ~7.1us of BSP preamble is fixed; DMA triggers can only issue after.
  - DMA: both HWDGE rings share ~420 GB/s and run ~210 GB/s for their
    first ~3.4us (cold clock). Ramp triggers are staggered so x_so[b0]
    chunk k0 is the FIRST sync-ring transfer (PE starts ~9.5us), with
    chunk group sizes growing 1,1,2,4,4,4 so per-wait stalls stay
    under ~0.5us (a multi-us PE gap re-gates the PE clock to 1.2GHz
    for 3.4us - that cost 5us in the previous schedule).
  - L1 weights are packed pair-interleaved [p, pair, k, j, m] so ONE
    scalar-ring trigger stream delivers (o0,o1) k-chunks in exactly the
    order the DMA-paced ramp consumes them: 4 PSUM groups (j,m) open,
    k-chunks interleaved, 4 matmuls + 211KB per 0.5us == the warm DMA
    delivery rate.
  - PE pre-warm: dummy matmuls (ungated - they read uninitialized SBUF,
    discarded) fill the PE queue from engine-init (~6us) to k0 arrival;
    the HAM clock gate needs ~3.4us of activity to reach 2.4GHz.
  - Steady batches (b1-b3) use the o-major loop (2-deep PSUM slack vs
    activation latency) and one x DMA trigger per tensor, prefetched
    2 batches deep; all input DMA lands by ~60us of a ~160us kernel.
Per batch element: x[b] is [2048, 588] (channels on SBUF partitions in
K-chunks of 128, positions on the free dim, 2 m-tiles of 294 = one PSUM
bank each). Channel->partition mapping is interleaved (partition p holds
channels 16p..16p+15) so every DMA descriptor is one contiguous
per-partition run; weights are permuted host-side to match.
"""

import numpy as np

import concourse.bass as bass  # noqa: F401
import concourse.tile as tile
from concourse import bacc, mybir
from concourse.bass_utils import run_bass_kernel_spmd

F16 = mybir.dt.float16
F32 = mybir.dt.float32

B, C, NN, HW = 32, 2048, 12, 49
NHW = NN * HW            # 588
N_CORES = 8
BPC = B // N_CORES       # 4 batch elements per core
MT = NHW // 2            # 294 columns = one PSUM bank of fp32
KC1 = C // 128           # 16 K-chunks, layer 1
OC1 = 512 // 128         # 4 output chunks, layer 1 (per branch)
KC2 = 1024 // 128        # 8 K-chunks, layer 2
OC2 = 256 // 128         # 2 output chunks, layer 2
EPS = 1e-5
SLOPE = 0.2
N_DUMMY = 28             # PE pre-warm matmuls (~107ns each at 1.2GHz)


def _fold_params(d):
    """Fold BNs into linears, in fp64. Returns device-layout arrays."""
    g = {k: np.asarray(v, dtype=np.float64) for k, v in d.items()}

    def bn_st(p):
        s = g[f"{p}_g"] / np.sqrt(g[f"{p}_v"] + EPS)
        t = g[f"{p}_b"] - g[f"{p}_m"] * s
        return s, t

    s_so, t_so = bn_st("bn_so")
    s_c, t_c = bn_st("bn_c")
    s1, t1 = bn_st("bn1")
    s2, t2 = bn_st("bn2")

    A_so = g["W_so"] * s_so[None, :]                 # [512, 2048]
    a_so = g["W_so"] @ t_so + g["b_so"]              # [512]
    A_c = g["W_c"] * s_c[None, :]
    a_c = g["W_c"] @ t_c + g["b_c"]
    A1 = s2[:, None] * (g["W1"] * s1[None, :])       # [256, 1024]
    a1 = s2 * (g["W1"] @ t1 + g["b1"]) + t2          # [256]

    # layer-1 weights, pair-interleaved: W_dev[p, P, k, j, m] =
    # A[128*(2P+j)+m, 16p+k]. Chunk k holds channel 16p+k at partition p
    # (matching the x DMA layout); within an o-pair P the k-chunks of
    # both halves j are adjacent, so a single address-ordered DMA
    # delivers them in ramp consumption order.
    def l1_prep(A):  # [512, 2048] -> [128, 2*16*2*128] fp16
        A6 = A.reshape(2, 2, 128, 128, KC1)          # [P, j, m, p, k]
        return np.ascontiguousarray(
            A6.transpose(3, 0, 4, 1, 2).reshape(128, 2 * KC1 * 2 * 128)
        ).astype(np.float16)

    wso = l1_prep(A_so)
    wc = l1_prep(A_c)
    # layer-2: W1_dev[p, o, k, m] = A1[128o+m, 128k+p] (f stores channel
    # 128k+p at partition p of column-block k).
    A4 = A1.reshape(OC2, 128, KC2, 128)              # [o, m, k, p]
    w1 = np.ascontiguousarray(
        A4.transpose(3, 0, 2, 1).reshape(128, OC2 * KC2 * 128)
    ).astype(np.float16)
    w2 = np.ascontiguousarray(g["W2"].reshape(OC2, 128).T).astype(np.float16)
    # bias pack [128, 10] fp32: bso(4) | bc(4) | b1(2)
    bias = np.concatenate([
        a_so.reshape(OC1, 128).T, a_c.reshape(OC1, 128).T,
        a1.reshape(OC2, 128).T], axis=1)
    bias = np.ascontiguousarray(bias).astype(np.float32)
    thresh = float(-HW * g["b2"][0])
    return wso, wc, w1, w2, bias, thresh


def _widx(P, k, j=0):
    """flat column of the pair-interleaved layer-1 weight block."""
    return ((P * KC1 + k) * 2 + j) * 128


RC = 2 * 128 + NHW       # 844 = one ramp chunk: wso-P0 (j0,j1) + x_so cols


def build_bass(thresh, repeat=1, loop=1):
    nc = bacc.Bacc("TRN2", target_bir_lowering=False, debug=False)

    xso_d = nc.dram_tensor("x_so", [BPC, C, NHW], F16, kind="ExternalInput").ap()
    xc_d = nc.dram_tensor("x_c", [BPC, C, NHW], F16, kind="ExternalInput").ap()
    # ramp tensor: b0's so-branch, k-chunks of (wso pair0 | x_so[b0]),
    # interleaved host-side so ONE sync-ring trigger stream delivers the
    # 4-group ramp's data in exact consumption order with 8 semaphores.
    ramp_d = nc.dram_tensor("ramp", [128, KC1 * RC], F16, kind="ExternalInput").ap()
    wso_d = nc.dram_tensor("wso", [128, OC1 * KC1 * 128], F16, kind="ExternalInput").ap()
    wc_d = nc.dram_tensor("wc", [128, OC1 * KC1 * 128], F16, kind="ExternalInput").ap()
    w1_d = nc.dram_tensor("w1", [128, OC2 * KC2 * 128], F16, kind="ExternalInput").ap()
    w2_d = nc.dram_tensor("w2", [128, OC2], F16, kind="ExternalInput").ap()
    bias_d = nc.dram_tensor("bias", [128, 2 * OC1 + OC2], F32, kind="ExternalInput").ap()
    out_d = nc.dram_tensor("out", [BPC * NN], F32, kind="ExternalOutput").ap()

    with tile.TileContext(nc) as tc:
        with (
            tc.tile_pool(name="wp", bufs=1) as wp,
            tc.tile_pool(name="xp", bufs=3) as xp,
            tc.tile_pool(name="fp", bufs=2) as fp,
            tc.tile_pool(name="hp", bufs=2) as hp,
            tc.tile_pool(name="ap", bufs=1) as ac,
            tc.tile_pool(name="ps1", bufs=4, space="PSUM") as ps1,
            tc.tile_pool(name="ps2", bufs=2, space="PSUM") as ps2,
            tc.tile_pool(name="ps3", bufs=2, space="PSUM") as ps3,
        ):
            # ---- tiny tensors on the gpsimd (SWDGE) ring ----
            # memset first: it gates the PE pre-warm matmuls.
            dummy_sb = wp.tile([128, 128], F16)
            nc.gpsimd.memset(dummy_sb[:], 0)
            bias_sb = wp.tile([128, 2 * OC1 + OC2], F32)
            nc.gpsimd.dma_start(bias_sb[:], bias_d[:])
            w2_sb = wp.tile([128, OC2], F16)
            nc.gpsimd.dma_start(w2_sb[:], w2_d[:])

            # weight SBUF tiles (DMAs are issued inside _body, on the
            # scalar HWDGE ring, staged in consumption order)
            wso_sb = wp.tile([128, OC1 * KC1 * 128], F16)
            wc_sb = wp.tile([128, OC1 * KC1 * 128], F16)
            w1_sb = wp.tile([128, OC2 * KC2 * 128], F16)
            ramp_sb = wp.tile([128, KC1 * RC], F16)

            # ---- PE pre-warm: HAM flips to 2.4GHz after ~3.4us of
            # activity; burn the DMA lead-in on dummy matmuls so the
            # real stream runs warm from the first real k-chunk.
            wps = ps1.tile([128, MT], F32, tag="ps1")
            for i in range(N_DUMMY):
                nc.tensor.matmul(wps[:, 0:128], lhsT=dummy_sb[:],
                                 rhs=dummy_sb[:], start=True, stop=True)

            out_sb = ac.tile([1, BPC * NN], F32)
            bits_sb = ac.tile([1, BPC * NN], F32)

            import contextlib
            loop_cm = tc.For_i(0, loop, 1) if loop > 1 else contextlib.nullcontext()
            with loop_cm:
                _body(nc, tc, repeat, xso_d, xc_d, ramp_d, out_d,
                      (wso_d, wc_d, w1_d), wso_sb, wc_sb, w1_sb, w2_sb,
                      ramp_sb, bias_sb, out_sb, bits_sb, xp, fp, hp,
                      ps1, ps2, ps3, thresh)

    nc.compile()
    return nc


def _body(nc, tc, repeat, xso_d, xc_d, ramp_d, out_d,
          weight_dram, wso_sb, wc_sb, w1_sb, w2_sb, ramp_sb, bias_sb,
          out_sb, bits_sb, xp, fp, hp, ps1, ps2, ps3, thresh):
    wso_d, wc_d, w1_d = weight_dram

    def x_sub(ring, x_sb, x_d, b, lo, hi):
        """DMA k-chunks [lo, hi) of x[b]."""
        x_t = x_sb.rearrange("p (j m) -> p j m", j=KC1)
        x_v = x_d[b].rearrange("(p j) m -> p j m", p=128)
        ring.dma_start(x_t[:, lo:hi, :], x_v[:, lo:hi, :])

    def w_sub(ring, sb, dr, P, lo, hi):
        """DMA k-chunks [lo, hi) of o-pair P of a layer-1 weight."""
        ring.dma_start(sb[:, _widx(P, lo):_widx(P, hi)],
                       dr[:, _widx(P, lo):_widx(P, hi)])

    def x_load(x_d, b, tag):
        """steady-state x load: one sync-ring trigger for the tensor.
        (scalar-ring triggers would queue behind earlier batches'
        activations; granularity is irrelevant - data lands ~30us
        before first use)."""
        x_sb = xp.tile([128, KC1 * NHW], F16, tag=tag)
        x_sub(nc.sync, x_sb, x_d, b, 0, KC1)
        return x_sb

    # the combined (wso-P0 | x_so[b0]) ramp tile is persistent (wp pool)
    # - steady batches read wso pair0 weights from it. x_load's "xso"
    # ring then has exactly 3 allocations (b1..b3): never recycled, so
    # no x trigger ever waits; "xc" recycles only b3 into b0's slot.
    ramp_t = ramp_sb.rearrange("p (k c) -> p k c", k=KC1)

    def l1_lhsT(br, o):
        P, j = divmod(o, 2)
        if br == 0 and P == 0:
            return lambda k: ramp_t[:, k, j * 128:(j + 1) * 128]
        w_sb = wso_sb if br == 0 else wc_sb
        return lambda k: w_sb[:, _widx(P, k, j):_widx(P, k, j) + 128]

    def l1_group(ps, lhsT_of, x_sb, m):
        for k in range(KC1):
            nc.tensor.matmul(
                ps[:], lhsT=lhsT_of(k),
                rhs=x_sb[:, k * NHW + m * MT:k * NHW + m * MT + MT],
                start=(k == 0), stop=(k == KC1 - 1))

    def l1_act(ps, f_sb, br, o, m, boff):
        col = (br * OC1 + o) * NHW + m * MT
        nc.scalar.activation(
            f_sb[:, col:col + MT], ps[:],
            mybir.ActivationFunctionType.Prelu,
            bias=bias_sb[:, boff + o:boff + o + 1], scale=1.0, alpha=SLOPE)

    def l1_pair_4g(f_sb, br, P, boff, rhs_of, post_act=None):
        """one o-pair with 4 PSUM groups (j,m) open, k-chunks
        interleaved: consumes each arriving chunk (x 147KB + w 64KB)
        with 4 matmuls (0.5us) - matches the warm DMA delivery rate.
        post_act() is emitted on the scalar queue right after the first
        activation - used to delay non-critical weight DMA triggers so
        they don't steal cold-window bandwidth from the ramp stream."""
        lhs = (l1_lhsT(br, 2 * P), l1_lhsT(br, 2 * P + 1))
        tiles = {}
        for j in range(2):
            for m in range(2):
                ramp_ps = ps1.tile([128, MT], F32, tag="ps1")
                tiles[(j, m)] = ramp_ps
        for k in range(KC1):
            for j in range(2):
                for m in range(2):
                    nc.tensor.matmul(
                        tiles[(j, m)][:], lhsT=lhs[j](k), rhs=rhs_of(k, m),
                        start=(k == 0), stop=(k == KC1 - 1))
        for j in range(2):
            for m in range(2):
                l1_act(tiles[(j, m)], f_sb, br, 2 * P + j, m, boff)

    for _rep in range(repeat):
        for b in range(BPC):
            first = _rep == 0 and b == 0
            if first:
                # ---- ramp trigger schedule ----
                # sync ring, in consumption order: the combined
                # (wso-P0 | x_so[b0]) tensor in 8 staggered triggers
                # (fine-grained during the DMA cold-clock window) - all
                # 8 HWDGE semaphores used at most once here, so no
                # trigger waits on an earlier transfer; then wso pair1
                # (k-ordered, feeds the pair1 4-group) and xc.
                ramp_v = ramp_d.rearrange("p (k c) -> p k c", k=KC1)
                for lo, hi in ((0, 1), (1, 2), (2, 3), (3, 4), (4, 6),
                               (6, 8), (8, 11), (11, 16)):
                    nc.sync.dma_start(ramp_t[:, lo:hi, :],
                                      ramp_v[:, lo:hi, :])
                w_sub(nc.sync, wso_sb, wso_d, 1, 0, KC1)
                xc_sb = xp.tile([128, KC1 * NHW], F16, tag="xc")
                x_sub(nc.sync, xc_sb, xc_d, b, 0, 8)
                x_sub(nc.sync, xc_sb, xc_d, b, 8, 16)
                # wc/w1 on the scalar ring, DELAYED via the scheduler's
                # manual timing override so they don't steal cold-window
                # DMA bandwidth from the ramp stream (the scheduler
                # hoists dependency-free triggers to the queue front
                # otherwise). Times are scheduler-sim ms.
                with tc.tile_wait_until(0.013):
                    w_sub(nc.scalar, wc_sb, wc_d, 0, 0, KC1)
                with tc.tile_wait_until(0.016):
                    w_sub(nc.scalar, wc_sb, wc_d, 1, 0, KC1)
                with tc.tile_wait_until(0.019):
                    nc.scalar.dma_start(w1_sb[:], w1_d[:])
            else:
                xso_sb = x_load(xso_d, b, "xso")
                xc_sb = x_load(xc_d, b, "xc")

            # ---- layer 1: f = lrelu(A @ x + a), fp16 out ----
            f_sb = fp.tile([128, 2 * OC1 * NHW], F16, tag="f")
            if first:
                # DMA-paced 4-group schedule for all four o-pairs; x of
                # the so-branch lives in the combined ramp tile.
                rx = lambda k, m: ramp_t[:, k, 256 + m * MT:256 + (m + 1) * MT]
                rxc = lambda k, m: xc_sb[:, k * NHW + m * MT:
                                         k * NHW + m * MT + MT]
                for P in range(2):
                    l1_pair_4g(f_sb, 0, P, 0, rx)
                for P in range(2):
                    l1_pair_4g(f_sb, 1, P, OC1, rxc)
            else:
                for br, (x_sb, boff) in enumerate(
                        ((xso_sb, 0), (xc_sb, OC1))):
                    for o in range(OC1):
                        for m in range(2):
                            ps = ps1.tile([128, MT], F32, tag="ps1")
                            l1_group(ps, l1_lhsT(br, o), x_sb, m)
                            l1_act(ps, f_sb, br, o, m, boff)

            # ---- layer 2: h = lrelu(A1 @ f + a1); layer 3 + mean-reduce:
            # y = W2 @ h ; sum 49-groups. For the last batch, L3-m runs
            # right after L2-m so only L3-m1 remains on the tail.
            h_sb = hp.tile([128, OC2 * NHW], F16, tag="h")
            last = b == BPC - 1
            for m in range(2):
                # last batch, m1: o1 first so act(o1) runs under o0's
                # matmuls; L3 then accumulates q1 before q0 (exact - a
                # two-term fp32 add commutes), leaving only the o0 act
                # on the tail's critical path.
                o_order = (1, 0) if (last and m == 1) else (0, 1)
                for o in o_order:
                    ps = ps2.tile([128, MT], F32, tag="ps2")
                    for k in range(KC2):
                        nc.tensor.matmul(
                            ps[:],
                            lhsT=w1_sb[:, (o * KC2 + k) * 128:
                                       (o * KC2 + k) * 128 + 128],
                            rhs=f_sb[:, k * NHW + m * MT:
                                     k * NHW + m * MT + MT],
                            start=(k == 0), stop=(k == KC2 - 1))
                    col = o * NHW + m * MT
                    nc.scalar.activation(
                        h_sb[:, col:col + MT], ps[:],
                        mybir.ActivationFunctionType.Prelu,
                        bias=bias_sb[:, 2 * OC1 + o:2 * OC1 + o + 1],
                        scale=1.0, alpha=SLOPE)
                for m3 in ([m] if last else ([0, 1] if m == 1 else [])):
                    ps = ps3.tile([1, MT], F32, tag="ps3")
                    q_order = (1, 0) if (last and m3 == 1) else (0, 1)
                    for qi, q in enumerate(q_order):
                        nc.tensor.matmul(
                            ps[:],
                            lhsT=w2_sb[:, q:q + 1],
                            rhs=h_sb[:, q * NHW + m3 * MT:
                                     q * NHW + m3 * MT + MT],
                            start=(qi == 0), stop=(qi == OC2 - 1))
                    off = b * NN + m3 * (MT // HW)
                    nc.vector.reduce_sum(
                        out_sb[0:1, off:off + MT // HW],
                        ps.rearrange("p (g x) -> p g x", x=HW),
                        axis=mybir.AxisListType.X)

            # ---- threshold + store, per batch (hides all but the last
            # ~2us under the next batch's compute):
            # sigmoid(mean) > 0.5  <=>  sum > -49*b2
            nc.vector.tensor_scalar(
                bits_sb[0:1, b * NN:(b + 1) * NN],
                out_sb[0:1, b * NN:(b + 1) * NN], float(thresh), None,
                mybir.AluOpType.is_gt)
            # b<last: gpsimd (SWDGE) ring - a sync-ring store would block
            # the later x-transfer triggers queued behind it until this
            # batch's whole compute chain finishes. Last batch: sync ring
            # (HWDGE completes ~0.6us faster, and nothing queues after).
            ring_out = nc.sync if last else nc.gpsimd
            ring_out.dma_start(out_d[b * NN:(b + 1) * NN],
                               bits_sb[0:1, b * NN:(b + 1) * NN])


_CACHE = {}


def _get_nc(thresh, repeat=1, loop=1):
    key = (round(thresh, 9), repeat, loop)
    if key not in _CACHE:
        _CACHE[key] = build_bass(thresh, repeat, loop)
    return _CACHE[key]


def _prepare(inputs):
    """Fold params, cast x to fp16, build per-core input maps + nc."""
    wso, wc, w1, w2, bias, thresh = _fold_params(inputs)
    xso = np.asarray(inputs["x_so"], dtype=np.float32).reshape(
        B, C, NHW).astype(np.float16)
    xc = np.asarray(inputs["x_c"], dtype=np.float32).reshape(
        B, C, NHW).astype(np.float16)
    wso_p0 = wso[:, :2 * KC1 * 128].reshape(128, KC1, 256)
    in_maps = []
    for i in range(N_CORES):
        # combined ramp tensor: per k-chunk, wso pair0 (256 cols) then
        # x_so[b0] (588 cols) - the ramp stream in consumption order.
        x0 = xso[i * BPC].reshape(128, KC1, NHW)
        ramp = np.concatenate([wso_p0, x0], axis=2).reshape(128, KC1 * RC)
        in_maps.append({
            "x_so": xso[i * BPC:(i + 1) * BPC],
            "x_c": xc[i * BPC:(i + 1) * BPC],
            "ramp": np.ascontiguousarray(ramp),
            "wso": wso, "wc": wc, "w1": w1, "w2": w2, "bias": bias,
        })
    return _get_nc(thresh), in_maps


def kernel(**inputs):
    nc, in_maps = _prepare(inputs)
    res = run_bass_kernel_spmd(nc, in_maps, list(range(N_CORES)))
    out = np.concatenate([res.results[i]["out"].reshape(BPC, NN)
                          for i in range(N_CORES)], axis=0)
    return np.ascontiguousarray(out.reshape(B, NN, 1).astype(np.float32))


# revision 25
# speedup vs baseline: 1.0199x; 1.0199x over previous
"""Trainium2 Bass kernel for nn_ContextGatingSigmoidClassifier.

Math (eval mode):
  f_so = lrelu(W_so @ bn_so(x_so) + b_so)        x: [B,2048,N,H,W]
  f_c  = lrelu(W_c  @ bn_c(x_c)  + b_c)
  f    = concat -> bn1 -> W1 -> bn2 -> lrelu -> W2 -> mean(H,W) -> sigmoid > 0.5

All BatchNorms are eval-mode affine maps, so they fold into the adjacent
linear layers (done host-side in fp64). Final threshold:
  sigmoid(mean) > 0.5  <=>  sum_hw(W2 @ h) > -49*b2.

Device mapping: data-parallel over batch (4 per core, 8 cores), weights
replicated, x cast fp32->fp16 host-side. The kernel is tensor-engine
bound (1152 matmuls of N=294 at 125ns = 144us); the schedule exists to
keep the PE stream dense from the first possible cycle. Measured HW
facts that shape it:
  - ~7.1us of fixed BSP preamble before any kernel instruction.
  - Both HWDGE rings share ~420 GB/s; each transfer costs ~0.6-1us of
    fixed overhead (trigger + descgen), so the early window delivers
    only ~150-250 GB/s with small transfers.
  - A PE idle gap of more than ~2-3us re-gates the PE clock to 1.2GHz
    for 3.4us+ (HAM) - stalls snowball. So batch 0's layer 1 runs an
    8-PSUM-group schedule (all 4 o-chunks x 2 m-tiles at once): each
    k-chunk of data (x 147KB + weights 128KB) feeds 8 matmuls (1us),
    so PE demand (~275 GB/s) roughly matches even the cold DMA rate
    and per-chunk waits stay well under the HAM threshold.
  - Batch 0's so-branch data (all 4 weight o-chunks + x) is packed
    host-side into ONE k-ordered stream, split across both rings in
    alternating, geometrically growing transfers - first 8 transfers
    use the 8 HWDGE semaphores at most once, so no trigger ever waits
    (the Tile scheduler recycles semaphores round-robin and a recycled
    trigger blocks on the previous user's completion).
  - wc/w1 triggers carry a manual scheduler delay (tile_wait_until) so
    they don't steal cold-window bandwidth - the scheduler hoists
    dependency-free triggers to the queue front otherwise.
  - PE pre-warm: dummy matmuls fill the PE queue from the preamble end
    (~7.4us) to the first k-chunk arrival (~10us).
  - Steady batches (b1-b3) use the o-major loop (2-deep PSUM slack vs
    activation latency) and one x DMA trigger per tensor, prefetched
    ahead; all input DMA lands by ~60us of a ~160us kernel.
Per batch element: x[b] is [2048, 588] (channels on SBUF partitions in
K-chunks of 128, positions on the free dim, 2 m-tiles of 294 = one PSUM
bank each). Channel->partition mapping is interleaved (partition p holds
channels 16p..16p+15) so every DMA descriptor is one contiguous
per-partition run; weights are permuted host-side to match.
"""

import numpy as np

import concourse.bass as bass  # noqa: F401
import concourse.tile as tile
from concourse import bacc, mybir
from concourse.bass_utils import run_bass_kernel_spmd

F16 = mybir.dt.float16
F32 = mybir.dt.float32

B, C, NN, HW = 32, 2048, 12, 49
NHW = NN * HW            # 588
N_CORES = 8
BPC = B // N_CORES       # 4 batch elements per core
MT = NHW // 2            # 294 columns = one PSUM bank of fp32
KC1 = C // 128           # 16 K-chunks, layer 1
OC1 = 512 // 128         # 4 output chunks, layer 1 (per branch)
KC2 = 1024 // 128        # 8 K-chunks, layer 2
OC2 = 256 // 128         # 2 output chunks, layer 2
EPS = 1e-5
SLOPE = 0.2
N_DUMMY = 28             # PE pre-warm matmuls (~107ns each at 1.2GHz)
RC = OC1 * 128 + NHW     # 1100 = one so-stream chunk: wso (4 o) + x cols


def _fold_params(d):
    """Fold BNs into linears, in fp64. Returns device-layout arrays."""
    g = {k: np.asarray(v, dtype=np.float64) for k, v in d.items()}

    def bn_st(p):
        s = g[f"{p}_g"] / np.sqrt(g[f"{p}_v"] + EPS)
        t = g[f"{p}_b"] - g[f"{p}_m"] * s
        return s, t

    s_so, t_so = bn_st("bn_so")
    s_c, t_c = bn_st("bn_c")
    s1, t1 = bn_st("bn1")
    s2, t2 = bn_st("bn2")

    A_so = g["W_so"] * s_so[None, :]                 # [512, 2048]
    a_so = g["W_so"] @ t_so + g["b_so"]              # [512]
    A_c = g["W_c"] * s_c[None, :]
    a_c = g["W_c"] @ t_c + g["b_c"]
    A1 = s2[:, None] * (g["W1"] * s1[None, :])       # [256, 1024]
    a1 = s2 * (g["W1"] @ t1 + g["b1"]) + t2          # [256]

    # so-weights, k-major quad-interleaved: Wq[p, k, o, m] =
    # A[128o+m, 16p+k]. Chunk k holds channel 16p+k at partition p
    # (matching the x DMA layout); all 4 o-chunks of a k sit together,
    # so the combined (weights|x) stream arrives in exactly the order
    # the 8-group schedule consumes it.
    A4 = A_so.reshape(OC1, 128, 128, KC1)            # [o, m, p, k]
    wq = np.ascontiguousarray(A4.transpose(2, 3, 0, 1))  # [p, k, o, m]
    wq = wq.reshape(128, KC1, OC1 * 128).astype(np.float16)
    # c-weights, pair-interleaved: W[p, P, k, j, m] = A[128*(2P+j)+m,
    # 16p+k] - both halves of an o-pair arrive k-ordered in one DMA.
    A6 = A_c.reshape(2, 2, 128, 128, KC1)            # [P, j, m, p, k]
    wc = np.ascontiguousarray(
        A6.transpose(3, 0, 4, 1, 2).reshape(128, 2 * KC1 * 2 * 128)
    ).astype(np.float16)
    # layer-2: W1_dev[p, o, k, m] = A1[128o+m, 128k+p] (f stores channel
    # 128k+p at partition p of column-block k).
    A4 = A1.reshape(OC2, 128, KC2, 128)              # [o, m, k, p]
    w1 = np.ascontiguousarray(
        A4.transpose(3, 0, 2, 1).reshape(128, OC2 * KC2 * 128)
    ).astype(np.float16)
    w2 = np.ascontiguousarray(g["W2"].reshape(OC2, 128).T).astype(np.float16)
    # bias pack [128, 10] fp32: bso(4) | bc(4) | b1(2)
    bias = np.concatenate([
        a_so.reshape(OC1, 128).T, a_c.reshape(OC1, 128).T,
        a1.reshape(OC2, 128).T], axis=1)
    bias = np.ascontiguousarray(bias).astype(np.float32)
    thresh = float(-HW * g["b2"][0])
    return wq, wc, w1, w2, bias, thresh


def _widx(P, k, j=0):
    """flat column of the pair-interleaved c-branch weight block."""
    return ((P * KC1 + k) * 2 + j) * 128


def build_bass(thresh, repeat=1, loop=1):
    nc = bacc.Bacc("TRN2", target_bir_lowering=False, debug=False)

    xso_d = nc.dram_tensor("x_so", [BPC, C, NHW], F16, kind="ExternalInput").ap()
    xc_d = nc.dram_tensor("x_c", [BPC, C, NHW], F16, kind="ExternalInput").ap()
    # combined so-stream: k-chunks of (all wso o-blocks | x_so[b0]),
    # interleaved host-side - the whole so-branch working set arrives
    # in consumption order.
    ramp_d = nc.dram_tensor("ramp", [128, KC1 * RC], F16, kind="ExternalInput").ap()
    wc_d = nc.dram_tensor("wc", [128, OC1 * KC1 * 128], F16, kind="ExternalInput").ap()
    w1_d = nc.dram_tensor("w1", [128, OC2 * KC2 * 128], F16, kind="ExternalInput").ap()
    w2_d = nc.dram_tensor("w2", [128, OC2], F16, kind="ExternalInput").ap()
    bias_d = nc.dram_tensor("bias", [128, 2 * OC1 + OC2], F32, kind="ExternalInput").ap()
    out_d = nc.dram_tensor("out", [BPC * NN], F32, kind="ExternalOutput").ap()

    with tile.TileContext(nc) as tc:
        with (
            tc.tile_pool(name="wp", bufs=1) as wp,
            tc.tile_pool(name="xp", bufs=3) as xp,
            tc.tile_pool(name="fp", bufs=2) as fp,
            tc.tile_pool(name="hp", bufs=2) as hp,
            tc.tile_pool(name="ap", bufs=1) as ac,
            tc.tile_pool(name="ps1", bufs=4, space="PSUM") as ps1,
            tc.tile_pool(name="ps2", bufs=2, space="PSUM") as ps2,
            tc.tile_pool(name="ps3", bufs=2, space="PSUM") as ps3,
        ):
            # ---- tiny tensors on the gpsimd (SWDGE) ring ----
            # memset first: it gates the PE pre-warm matmuls.
            dummy_sb = wp.tile([128, 128], F16)
            nc.gpsimd.memset(dummy_sb[:], 0)
            bias_sb = wp.tile([128, 2 * OC1 + OC2], F32)
            nc.gpsimd.dma_start(bias_sb[:], bias_d[:])
            w2_sb = wp.tile([128, OC2], F16)
            nc.gpsimd.dma_start(w2_sb[:], w2_d[:])

            # weight SBUF tiles (DMAs are issued inside _body, staged in
            # consumption order); comb_sb holds the combined so-stream
            # (weights stay resident for b1-b3).
            comb_sb = wp.tile([128, KC1 * RC], F16)
            wc_sb = wp.tile([128, OC1 * KC1 * 128], F16)
            w1_sb = wp.tile([128, OC2 * KC2 * 128], F16)

            # ---- PE pre-warm: HAM flips to 2.4GHz after ~3.4us of
            # activity; burn the DMA lead-in on dummy matmuls so the
            # real stream runs warm from the first real k-chunk.
            wps = ps1.tile([128, MT], F32, tag="ps1")
            for i in range(N_DUMMY):
                nc.tensor.matmul(wps[:, 0:128], lhsT=dummy_sb[:],
                                 rhs=dummy_sb[:], start=True, stop=True)

            out_sb = ac.tile([1, BPC * NN], F32)
            bits_sb = ac.tile([1, BPC * NN], F32)

            import contextlib
            loop_cm = tc.For_i(0, loop, 1) if loop > 1 else contextlib.nullcontext()
            with loop_cm:
                _body(nc, tc, repeat, xso_d, xc_d, ramp_d, out_d,
                      (wc_d, w1_d), comb_sb, wc_sb, w1_sb, w2_sb,
                      bias_sb, out_sb, bits_sb, xp, fp, hp,
                      ps1, ps2, ps3, thresh)

    nc.compile()
    return nc


def _body(nc, tc, repeat, xso_d, xc_d, ramp_d, out_d,
          weight_dram, comb_sb, wc_sb, w1_sb, w2_sb, bias_sb,
          out_sb, bits_sb, xp, fp, hp, ps1, ps2, ps3, thresh):
    wc_d, w1_d = weight_dram
    comb_t = comb_sb.rearrange("p (k c) -> p k c", k=KC1)
    XO = OC1 * 128           # x column offset inside a combined chunk

    def x_sub(ring, x_sb, x_d, b, lo, hi):
        """DMA k-chunks [lo, hi) of x[b]."""
        x_t = x_sb.rearrange("p (j m) -> p j m", j=KC1)
        x_v = x_d[b].rearrange("(p j) m -> p j m", p=128)
        ring.dma_start(x_t[:, lo:hi, :], x_v[:, lo:hi, :])

    def x_load(x_d, b, tag):
        """steady-state x load: one sync-ring trigger for the tensor.
        (scalar-ring triggers would queue behind earlier batches'
        activations; granularity is irrelevant - data lands ~30us
        before first use)."""
        x_sb = xp.tile([128, KC1 * NHW], F16, tag=tag)
        x_sub(nc.sync, x_sb, x_d, b, 0, KC1)
        return x_sb

    def l1_lhsT(br, o):
        if br == 0:
            return lambda k: comb_t[:, k, o * 128:(o + 1) * 128]
        P, j = divmod(o, 2)
        return lambda k: wc_sb[:, _widx(P, k, j):_widx(P, k, j) + 128]

    def l1_group(ps, lhsT_of, rhs_of, m):
        for k in range(KC1):
            nc.tensor.matmul(
                ps[:], lhsT=lhsT_of(k), rhs=rhs_of(k, m),
                start=(k == 0), stop=(k == KC1 - 1))

    def l1_act(ps, f_sb, br, o, m, boff):
        col = (br * OC1 + o) * NHW + m * MT
        nc.scalar.activation(
            f_sb[:, col:col + MT], ps[:],
            mybir.ActivationFunctionType.Prelu,
            bias=bias_sb[:, boff + o:boff + o + 1], scale=1.0, alpha=SLOPE)

    def l1_8g(f_sb, br, boff, rhs_of):
        """whole branch with 8 PSUM groups (o,m) open and k-chunks
        interleaved: each arriving chunk (x 147KB + w 128KB) feeds 8
        matmuls (1us), so PE demand matches even the cold DMA rate and
        per-chunk waits stay far below the HAM idle threshold."""
        lhs = [l1_lhsT(br, o) for o in range(OC1)]
        tiles = {}
        for o in range(OC1):
            pool = (ps1, ps1, ps2, ps3)[o]
            for m in range(2):
                g_ps = pool.tile([128, MT], F32, tag=("ps1", "ps1", "ps2",
                                                      "ps3")[o])
                tiles[(o, m)] = g_ps
        for k in range(KC1):
            for o in range(OC1):
                for m in range(2):
                    nc.tensor.matmul(
                        tiles[(o, m)][:], lhsT=lhs[o](k), rhs=rhs_of(k, m),
                        start=(k == 0), stop=(k == KC1 - 1))
        for o in range(OC1):
            for m in range(2):
                l1_act(tiles[(o, m)], f_sb, br, o, m, boff)

    for _rep in range(repeat):
        for b in range(BPC):
            first = _rep == 0 and b == 0
            if first:
                # ---- ramp trigger schedule ----
                # combined so-stream split across both rings in
                # alternating, growing transfers; with xc k0-7 these
                # use the 8 HWDGE semaphores exactly once (no trigger
                # waits). xc's tail and the delayed wc/w1 recycle
                # semaphores of long-completed transfers.
                ramp_v = ramp_d.rearrange("p (k c) -> p k c", k=KC1)
                rings = (nc.sync, nc.scalar)
                for i, (lo, hi) in enumerate(
                        ((0, 1), (1, 2), (2, 4), (4, 6),
                         (6, 9), (9, 12), (12, 16))):
                    rings[i % 2].dma_start(comb_t[:, lo:hi, :],
                                           ramp_v[:, lo:hi, :])
                xc_sb = xp.tile([128, KC1 * NHW], F16, tag="xc")
                x_sub(nc.scalar, xc_sb, xc_d, b, 0, 8)
                x_sub(nc.sync, xc_sb, xc_d, b, 8, 16)
                with tc.tile_wait_until(0.014):
                    nc.scalar.dma_start(
                        wc_sb[:, _widx(0, 0):_widx(1, 0)],
                        wc_d[:, _widx(0, 0):_widx(1, 0)])
                with tc.tile_wait_until(0.017):
                    nc.sync.dma_start(
                        wc_sb[:, _widx(1, 0):_widx(2, 0)],
                        wc_d[:, _widx(1, 0):_widx(2, 0)])
                with tc.tile_wait_until(0.020):
                    nc.scalar.dma_start(w1_sb[:], w1_d[:])
            else:
                xso_sb = x_load(xso_d, b, "xso")
                xc_sb = x_load(xc_d, b, "xc")

            # ---- layer 1: f = lrelu(A @ x + a), fp16 out ----
            f_sb = fp.tile([128, 2 * OC1 * NHW], F16, tag="f")
            if first:
                l1_8g(f_sb, 0, 0,
                      lambda k, m: comb_t[:, k, XO + m * MT:
                                          XO + (m + 1) * MT])
                l1_8g(f_sb, 1, OC1,
                      lambda k, m: xc_sb[:, k * NHW + m * MT:
                                         k * NHW + m * MT + MT])
            else:
                for br, (x_sb, boff) in enumerate(
                        ((xso_sb, 0), (xc_sb, OC1))):
                    rhs = lambda k, m, x_sb=x_sb: x_sb[
                        :, k * NHW + m * MT:k * NHW + m * MT + MT]
                    for o in range(OC1):
                        for m in range(2):
                            ps = ps1.tile([128, MT], F32, tag="ps1")
                            l1_group(ps, l1_lhsT(br, o), rhs, m)
                            l1_act(ps, f_sb, br, o, m, boff)

            # ---- layer 2: h = lrelu(A1 @ f + a1); layer 3 + mean-reduce:
            # y = W2 @ h ; sum 49-groups. For the last batch, L3-m runs
            # right after L2-m so only L3-m1 remains on the tail.
            h_sb = hp.tile([128, OC2 * NHW], F16, tag="h")
            last = b == BPC - 1
            for m in range(2):
                # last batch, m1: o1 first so act(o1) runs under o0's
                # matmuls; L3 then accumulates q1 before q0 (exact - a
                # two-term fp32 add commutes), leaving only the o0 act
                # on the tail's critical path.
                o_order = (1, 0) if (last and m == 1) else (0, 1)
                for o in o_order:
                    ps = ps2.tile([128, MT], F32, tag="ps2")
                    for k in range(KC2):
                        nc.tensor.matmul(
                            ps[:],
                            lhsT=w1_sb[:, (o * KC2 + k) * 128:
                                       (o * KC2 + k) * 128 + 128],
                            rhs=f_sb[:, k * NHW + m * MT:
                                     k * NHW + m * MT + MT],
                            start=(k == 0), stop=(k == KC2 - 1))
                    col = o * NHW + m * MT
                    nc.scalar.activation(
                        h_sb[:, col:col + MT], ps[:],
                        mybir.ActivationFunctionType.Prelu,
                        bias=bias_sb[:, 2 * OC1 + o:2 * OC1 + o + 1],
                        scale=1.0, alpha=SLOPE)
                for m3 in ([m] if last else ([0, 1] if m == 1 else [])):
                    ps = ps3.tile([1, MT], F32, tag="ps3")
                    q_order = (1, 0) if (last and m3 == 1) else (0, 1)
                    for qi, q in enumerate(q_order):
                        nc.tensor.matmul(
                            ps[:],
                            lhsT=w2_sb[:, q:q + 1],
                            rhs=h_sb[:, q * NHW + m3 * MT:
                                     q * NHW + m3 * MT + MT],
                            start=(qi == 0), stop=(qi == OC2 - 1))
                    off = b * NN + m3 * (MT // HW)
                    nc.vector.reduce_sum(
                        out_sb[0:1, off:off + MT // HW],
                        ps.rearrange("p (g x) -> p g x", x=HW),
                        axis=mybir.AxisListType.X)

            # ---- threshold + store, per batch (hides all but the last
            # ~2us under the next batch's compute):
            # sigmoid(mean) > 0.5  <=>  sum > -49*b2
            nc.vector.tensor_scalar(
                bits_sb[0:1, b * NN:(b + 1) * NN],
                out_sb[0:1, b * NN:(b + 1) * NN], float(thresh), None,
                mybir.AluOpType.is_gt)
            # b<last: gpsimd (SWDGE) ring - a sync-ring store would block
            # the later x-transfer triggers queued behind it until this
            # batch's whole compute chain finishes. Last batch: sync ring
            # (HWDGE completes ~0.6us faster, and nothing queues after).
            ring_out = nc.sync if last else nc.gpsimd
            ring_out.dma_start(out_d[b * NN:(b + 1) * NN],
                               bits_sb[0:1, b * NN:(b + 1) * NN])


_CACHE = {}


def _get_nc(thresh, repeat=1, loop=1):
    key = (round(thresh, 9), repeat, loop)
    if key not in _CACHE:
        _CACHE[key] = build_bass(thresh, repeat, loop)
    return _CACHE[key]


def _prepare(inputs):
    """Fold params, cast x to fp16, build per-core input maps + nc."""
    wq, wc, w1, w2, bias, thresh = _fold_params(inputs)
    xso = np.asarray(inputs["x_so"], dtype=np.float32).reshape(
        B, C, NHW).astype(np.float16)
    xc = np.asarray(inputs["x_c"], dtype=np.float32).reshape(
        B, C, NHW).astype(np.float16)
    in_maps = []
    for i in range(N_CORES):
        # combined so-stream: per k-chunk, all wso o-blocks (512 cols)
        # then x_so[b0] (588 cols) - b0's so data in consumption order.
        x0 = xso[i * BPC].reshape(128, KC1, NHW)
        ramp = np.concatenate([wq, x0], axis=2).reshape(128, KC1 * RC)
        in_maps.append({
            "x_so": xso[i * BPC:(i + 1) * BPC],
            "x_c": xc[i * BPC:(i + 1) * BPC],
            "ramp": np.ascontiguousarray(ramp),
            "wc": wc, "w1": w1, "w2": w2, "bias": bias,
        })
    return _get_nc(thresh), in_maps


def kernel(**inputs):
    nc, in_maps = _prepare(inputs)
    res = run_bass_kernel_spmd(nc, in_maps, list(range(N_CORES)))
    out = np.concatenate([res.results[i]["out"].reshape(BPC, NN)
                          for i in range(N_CORES)], axis=0)
    return np.ascontiguousarray(out.reshape(B, NN, 1).astype(np.float32))


# revision 26
# speedup vs baseline: 1.0345x; 1.0143x over previous
"""Trainium2 Bass kernel for nn_ContextGatingSigmoidClassifier.

Math (eval mode):
  f_so = lrelu(W_so @ bn_so(x_so) + b_so)        x: [B,2048,N,H,W]
  f_c  = lrelu(W_c  @ bn_c(x_c)  + b_c)
  f    = concat -> bn1 -> W1 -> bn2 -> lrelu -> W2 -> mean(H,W) -> sigmoid > 0.5

All BatchNorms are eval-mode affine maps, so they fold into the adjacent
linear layers (done host-side in fp64). Final threshold:
  sigmoid(mean) > 0.5  <=>  sum_hw(W2 @ h) > -49*b2.

Device mapping: data-parallel over batch (4 per core, 8 cores), weights
replicated, x cast fp32->fp16 host-side. The kernel is tensor-engine
bound (1152 matmuls of N=294 at 125ns = 144us); the schedule exists to
keep the PE stream dense from the first possible cycle. Measured HW
facts that shape it:
  - ~7.1us of fixed BSP preamble before any kernel instruction.
  - Both HWDGE rings share ~420 GB/s; each transfer costs ~0.6-1us of
    fixed overhead (trigger + descgen), and the first ~4us of DMA
    activity runs at roughly half rate (cold clock).
  - A PE idle gap of more than ~2-3us re-gates the PE clock to 1.2GHz
    for 3.4us+ (HAM) - stalls snowball. So batch 0 runs layer 1 with
    an 8-PSUM-group schedule (4 o-chunks x 2 m-tiles at once): each
    k-chunk of data (x 147KB + weights 128KB) feeds 8 matmuls (1us),
    so PE demand (~275 GB/s) roughly matches even the cold DMA rate
    and per-chunk waits stay well under the HAM threshold.
  - Each of batch 0's branches gets its whole working set (4 weight
    o-blocks + x, 4.4MB) packed host-side into ONE k-ordered stream,
    split across both rings in alternating, geometrically growing
    transfers. The first 8 transfers use the 8 HWDGE semaphores at
    most once, so no critical trigger ever waits (the Tile framework
    recycles semaphores round-robin and a recycled trigger blocks on
    the previous user's completion); later triggers' recycled waits
    land on long-completed transfers.
  - w1 and the b1-b3 x loads carry manual scheduler delays
    (tile_wait_until) so the scheduler cannot hoist these dependency-
    free triggers in front of the critical streams.
  - PE pre-warm: dummy matmuls fill the PE queue from the preamble end
    (~7.4us) to the first k-chunk arrival (~10us).
  - Steady batches (b1-b3) use the o-major loop (2-deep PSUM slack vs
    activation latency), reading layer-1 weights from the resident
    stream tiles; all input DMA lands by ~70us of a ~165us kernel.
Per batch element: x[b] is [2048, 588] (channels on SBUF partitions in
K-chunks of 128, positions on the free dim, 2 m-tiles of 294 = one PSUM
bank each). Channel->partition mapping is interleaved (partition p holds
channels 16p..16p+15) so every DMA descriptor is one contiguous
per-partition run; weights are permuted host-side to match.
"""

import numpy as np

import concourse.bass as bass  # noqa: F401
import concourse.tile as tile
from concourse import bacc, mybir
from concourse.bass_utils import run_bass_kernel_spmd

F16 = mybir.dt.float16
F32 = mybir.dt.float32

B, C, NN, HW = 32, 2048, 12, 49
NHW = NN * HW            # 588
N_CORES = 8
BPC = B // N_CORES       # 4 batch elements per core
MT = NHW // 2            # 294 columns = one PSUM bank of fp32
KC1 = C // 128           # 16 K-chunks, layer 1
OC1 = 512 // 128         # 4 output chunks, layer 1 (per branch)
KC2 = 1024 // 128        # 8 K-chunks, layer 2
OC2 = 256 // 128         # 2 output chunks, layer 2
EPS = 1e-5
SLOPE = 0.2
N_DUMMY = 28             # PE pre-warm matmuls (~107ns each at 1.2GHz)
XO = OC1 * 128           # x column offset inside a combined chunk
RC = XO + NHW            # 1100 = combined chunk: weights (4 o) + x cols


def _quad(A):
    """[512, 2048] -> [128, KC1, 512] k-major weight layout:
    Wq[p, k, o, m] = A[128o+m, 16p+k] (chunk k holds channel 16p+k at
    partition p, matching the x DMA layout)."""
    A4 = A.reshape(OC1, 128, 128, KC1)               # [o, m, p, k]
    return np.ascontiguousarray(
        A4.transpose(2, 3, 0, 1).reshape(128, KC1, OC1 * 128))


def _fold_params(d):
    """Fold BNs into linears, in fp64. Returns device-layout arrays."""
    g = {k: np.asarray(v, dtype=np.float64) for k, v in d.items()}

    def bn_st(p):
        s = g[f"{p}_g"] / np.sqrt(g[f"{p}_v"] + EPS)
        t = g[f"{p}_b"] - g[f"{p}_m"] * s
        return s, t

    s_so, t_so = bn_st("bn_so")
    s_c, t_c = bn_st("bn_c")
    s1, t1 = bn_st("bn1")
    s2, t2 = bn_st("bn2")

    A_so = g["W_so"] * s_so[None, :]                 # [512, 2048]
    a_so = g["W_so"] @ t_so + g["b_so"]              # [512]
    A_c = g["W_c"] * s_c[None, :]
    a_c = g["W_c"] @ t_c + g["b_c"]
    A1 = s2[:, None] * (g["W1"] * s1[None, :])       # [256, 1024]
    a1 = s2 * (g["W1"] @ t1 + g["b1"]) + t2          # [256]

    wso = _quad(A_so).astype(np.float16)
    wc = _quad(A_c).astype(np.float16)
    # layer-2: W1_dev[p, o, k, m] = A1[128o+m, 128k+p] (f stores channel
    # 128k+p at partition p of column-block k).
    A4 = A1.reshape(OC2, 128, KC2, 128)              # [o, m, k, p]
    w1 = np.ascontiguousarray(
        A4.transpose(3, 0, 2, 1).reshape(128, OC2 * KC2 * 128)
    ).astype(np.float16)
    w2 = np.ascontiguousarray(g["W2"].reshape(OC2, 128).T).astype(np.float16)
    # bias pack [128, 10] fp32: bso(4) | bc(4) | b1(2)
    bias = np.concatenate([
        a_so.reshape(OC1, 128).T, a_c.reshape(OC1, 128).T,
        a1.reshape(OC2, 128).T], axis=1)
    bias = np.ascontiguousarray(bias).astype(np.float32)
    thresh = float(-HW * g["b2"][0])
    return wso, wc, w1, w2, bias, thresh


def build_bass(thresh, repeat=1, loop=1):
    nc = bacc.Bacc("TRN2", target_bir_lowering=False, debug=False)

    xso_d = nc.dram_tensor("x_so", [BPC, C, NHW], F16, kind="ExternalInput").ap()
    xc_d = nc.dram_tensor("x_c", [BPC, C, NHW], F16, kind="ExternalInput").ap()
    # combined streams: k-chunks of (all weight o-blocks | x[b0]) per
    # branch - each branch's whole working set in consumption order.
    cso_d = nc.dram_tensor("comb_so", [128, KC1 * RC], F16, kind="ExternalInput").ap()
    cc_d = nc.dram_tensor("comb_c", [128, KC1 * RC], F16, kind="ExternalInput").ap()
    w1_d = nc.dram_tensor("w1", [128, OC2 * KC2 * 128], F16, kind="ExternalInput").ap()
    w2_d = nc.dram_tensor("w2", [128, OC2], F16, kind="ExternalInput").ap()
    bias_d = nc.dram_tensor("bias", [128, 2 * OC1 + OC2], F32, kind="ExternalInput").ap()
    out_d = nc.dram_tensor("out", [BPC * NN], F32, kind="ExternalOutput").ap()

    with tile.TileContext(nc) as tc:
        with (
            tc.tile_pool(name="wp", bufs=1) as wp,
            tc.tile_pool(name="xp", bufs=2) as xp,
            tc.tile_pool(name="fp", bufs=2) as fp,
            tc.tile_pool(name="hp", bufs=2) as hp,
            tc.tile_pool(name="ap", bufs=1) as ac,
            tc.tile_pool(name="ps1", bufs=4, space="PSUM") as ps1,
            tc.tile_pool(name="ps2", bufs=2, space="PSUM") as ps2,
            tc.tile_pool(name="ps3", bufs=2, space="PSUM") as ps3,
        ):
            # ---- tiny tensors on the gpsimd (SWDGE) ring ----
            # memset first: it gates the PE pre-warm matmuls.
            dummy_sb = wp.tile([128, 128], F16)
            nc.gpsimd.memset(dummy_sb[:], 0)
            bias_sb = wp.tile([128, 2 * OC1 + OC2], F32)
            nc.gpsimd.dma_start(bias_sb[:], bias_d[:])
            w2_sb = wp.tile([128, OC2], F16)
            nc.gpsimd.dma_start(w2_sb[:], w2_d[:])

            # combined-stream tiles stay resident: steady batches read
            # layer-1 weights from them.
            cso_sb = wp.tile([128, KC1 * RC], F16)
            cc_sb = wp.tile([128, KC1 * RC], F16)
            w1_sb = wp.tile([128, OC2 * KC2 * 128], F16)

            # ---- PE pre-warm: HAM flips to 2.4GHz after ~3.4us of
            # activity; burn the DMA lead-in on dummy matmuls so the
            # real stream runs warm from the first real k-chunk.
            wps = ps1.tile([128, MT], F32, tag="ps1")
            for i in range(N_DUMMY):
                nc.tensor.matmul(wps[:, 0:128], lhsT=dummy_sb[:],
                                 rhs=dummy_sb[:], start=True, stop=True)

            out_sb = ac.tile([1, BPC * NN], F32)
            bits_sb = ac.tile([1, BPC * NN], F32)

            import contextlib
            loop_cm = tc.For_i(0, loop, 1) if loop > 1 else contextlib.nullcontext()
            with loop_cm:
                _body(nc, tc, repeat, xso_d, xc_d, (cso_d, cc_d, w1_d),
                      out_d, (cso_sb, cc_sb), w1_sb, w2_sb,
                      bias_sb, out_sb, bits_sb, xp, fp, hp,
                      ps1, ps2, ps3, thresh)

    nc.compile()
    return nc


def _body(nc, tc, repeat, xso_d, xc_d, comb_dram, out_d,
          comb_sb, w1_sb, w2_sb, bias_sb,
          out_sb, bits_sb, xp, fp, hp, ps1, ps2, ps3, thresh):
    cso_d, cc_d, w1_d = comb_dram
    comb_t = [sb.rearrange("p (k c) -> p k c", k=KC1) for sb in comb_sb]
    comb_v = [dd.rearrange("p (k c) -> p k c", k=KC1) for dd in (cso_d, cc_d)]

    def x_sub(ring, x_sb, x_d, b, lo, hi):
        """DMA k-chunks [lo, hi) of x[b]."""
        x_t = x_sb.rearrange("p (j m) -> p j m", j=KC1)
        x_v = x_d[b].rearrange("(p j) m -> p j m", p=128)
        ring.dma_start(x_t[:, lo:hi, :], x_v[:, lo:hi, :])

    def l1_lhsT(br, o):
        return lambda k: comb_t[br][:, k, o * 128:(o + 1) * 128]

    def l1_act(ps, f_sb, br, o, m):
        col = (br * OC1 + o) * NHW + m * MT
        boff = br * OC1 + o
        nc.scalar.activation(
            f_sb[:, col:col + MT], ps[:],
            mybir.ActivationFunctionType.Prelu,
            bias=bias_sb[:, boff:boff + 1], scale=1.0, alpha=SLOPE)

    def l1_8g(f_sb, br, rhs_of):
        """whole branch with 8 PSUM groups (o,m) open and k-chunks
        interleaved: each arriving chunk (x 147KB + w 128KB) feeds 8
        matmuls (1us), so PE demand matches even the cold DMA rate and
        per-chunk waits stay far below the HAM idle threshold."""
        lhs = [l1_lhsT(br, o) for o in range(OC1)]
        pools = (ps1, ps1, ps2, ps3)
        tags = ("ps1", "ps1", "ps2", "ps3")
        tiles = {}
        for o in range(OC1):
            for m in range(2):
                g_ps = pools[o].tile([128, MT], F32, tag=tags[o])
                tiles[(o, m)] = g_ps
        for k in range(KC1):
            for o in range(OC1):
                for m in range(2):
                    nc.tensor.matmul(
                        tiles[(o, m)][:], lhsT=lhs[o](k), rhs=rhs_of(k, m),
                        start=(k == 0), stop=(k == KC1 - 1))
        for o in range(OC1):
            for m in range(2):
                l1_act(tiles[(o, m)], f_sb, br, o, m)

    for _rep in range(repeat):
        for b in range(BPC):
            first = _rep == 0 and b == 0
            if first:
                # ---- ramp trigger schedule ----
                # both branch streams alternate across the rings in
                # growing transfers; the first 8 triggers use the 8
                # HWDGE semaphores exactly once.
                rings = (nc.sync, nc.scalar)
                for i, (lo, hi) in enumerate(
                        ((0, 1), (1, 2), (2, 4), (4, 6),
                         (6, 9), (9, 12), (12, 16))):
                    rings[i % 2].dma_start(comb_t[0][:, lo:hi, :],
                                           comb_v[0][:, lo:hi, :])
                for i, (lo, hi) in enumerate(
                        ((0, 4), (4, 8), (8, 12), (12, 16))):
                    rings[(i + 1) % 2].dma_start(comb_t[1][:, lo:hi, :],
                                                 comb_v[1][:, lo:hi, :])
                with tc.tile_wait_until(0.018):
                    nc.scalar.dma_start(w1_sb[:], w1_d[:])
            else:
                # steady-state x loads: one sync-ring trigger per
                # tensor, scheduler-delayed so they queue behind the
                # batch-0 streams; data still lands ~30us before use.
                delay = 0.020 + 0.004 * (_rep * BPC + b)
                with tc.tile_wait_until(delay):
                    xso_sb = xp.tile([128, KC1 * NHW], F16, tag="xso")
                    x_sub(nc.sync, xso_sb, xso_d, b, 0, KC1)
                with tc.tile_wait_until(delay + 0.002):
                    xc_sb = xp.tile([128, KC1 * NHW], F16, tag="xc")
                    x_sub(nc.sync, xc_sb, xc_d, b, 0, KC1)

            # ---- layer 1: f = lrelu(A @ x + a), fp16 out ----
            f_sb = fp.tile([128, 2 * OC1 * NHW], F16, tag="f")
            if first:
                for br in range(2):
                    l1_8g(f_sb, br,
                          lambda k, m, br=br: comb_t[br][:, k,
                                                         XO + m * MT:
                                                         XO + (m + 1) * MT])
            else:
                for br, x_sb in enumerate((xso_sb, xc_sb)):
                    rhs = lambda k, m, x_sb=x_sb: x_sb[
                        :, k * NHW + m * MT:k * NHW + m * MT + MT]
                    for o in range(OC1):
                        for m in range(2):
                            ps = ps1.tile([128, MT], F32, tag="ps1")
                            for k in range(KC1):
                                nc.tensor.matmul(
                                    ps[:], lhsT=l1_lhsT(br, o)(k),
                                    rhs=rhs(k, m),
                                    start=(k == 0), stop=(k == KC1 - 1))
                            l1_act(ps, f_sb, br, o, m)

            # ---- layer 2: h = lrelu(A1 @ f + a1); layer 3 + mean-reduce:
            # y = W2 @ h ; sum 49-groups. For the last batch, L3-m runs
            # right after L2-m so only L3-m1 remains on the tail.
            h_sb = hp.tile([128, OC2 * NHW], F16, tag="h")
            last = b == BPC - 1
            for m in range(2):
                # last batch, m1: o1 first so act(o1) runs under o0's
                # matmuls; L3 then accumulates q1 before q0 (exact - a
                # two-term fp32 add commutes), leaving only the o0 act
                # on the tail's critical path.
                o_order = (1, 0) if (last and m == 1) else (0, 1)
                for o in o_order:
                    ps = ps2.tile([128, MT], F32, tag="ps2")
                    for k in range(KC2):
                        nc.tensor.matmul(
                            ps[:],
                            lhsT=w1_sb[:, (o * KC2 + k) * 128:
                                       (o * KC2 + k) * 128 + 128],
                            rhs=f_sb[:, k * NHW + m * MT:
                                     k * NHW + m * MT + MT],
                            start=(k == 0), stop=(k == KC2 - 1))
                    col = o * NHW + m * MT
                    nc.scalar.activation(
                        h_sb[:, col:col + MT], ps[:],
                        mybir.ActivationFunctionType.Prelu,
                        bias=bias_sb[:, 2 * OC1 + o:2 * OC1 + o + 1],
                        scale=1.0, alpha=SLOPE)
                for m3 in ([m] if last else ([0, 1] if m == 1 else [])):
                    ps = ps3.tile([1, MT], F32, tag="ps3")
                    q_order = (1, 0) if (last and m3 == 1) else (0, 1)
                    for qi, q in enumerate(q_order):
                        nc.tensor.matmul(
                            ps[:],
                            lhsT=w2_sb[:, q:q + 1],
                            rhs=h_sb[:, q * NHW + m3 * MT:
                                     q * NHW + m3 * MT + MT],
                            start=(qi == 0), stop=(qi == OC2 - 1))
                    off = b * NN + m3 * (MT // HW)
                    nc.vector.reduce_sum(
                        out_sb[0:1, off:off + MT // HW],
                        ps.rearrange("p (g x) -> p g x", x=HW),
                        axis=mybir.AxisListType.X)

            # ---- threshold + store, per batch (hides all but the last
            # ~2us under the next batch's compute):
            # sigmoid(mean) > 0.5  <=>  sum > -49*b2
            nc.vector.tensor_scalar(
                bits_sb[0:1, b * NN:(b + 1) * NN],
                out_sb[0:1, b * NN:(b + 1) * NN], float(thresh), None,
                mybir.AluOpType.is_gt)
            # b<last: gpsimd (SWDGE) ring - a sync-ring store would block
            # the later x-transfer triggers queued behind it until this
            # batch's whole compute chain finishes. Last batch: sync ring
            # (HWDGE completes ~0.6us faster, and nothing queues after).
            ring_out = nc.sync if last else nc.gpsimd
            ring_out.dma_start(out_d[b * NN:(b + 1) * NN],
                               bits_sb[0:1, b * NN:(b + 1) * NN])


_CACHE = {}


def _get_nc(thresh, repeat=1, loop=1):
    key = (round(thresh, 9), repeat, loop)
    if key not in _CACHE:
        _CACHE[key] = build_bass(thresh, repeat, loop)
    return _CACHE[key]


def _prepare(inputs):
    """Fold params, cast x to fp16, build per-core input maps + nc."""
    wso, wc, w1, w2, bias, thresh = _fold_params(inputs)
    xso = np.asarray(inputs["x_so"], dtype=np.float32).reshape(
        B, C, NHW).astype(np.float16)
    xc = np.asarray(inputs["x_c"], dtype=np.float32).reshape(
        B, C, NHW).astype(np.float16)
    in_maps = []
    for i in range(N_CORES):
        # combined streams: per k-chunk, all weight o-blocks (512 cols)
        # then x[b0] (588 cols) - b0's data in consumption order.
        cso = np.concatenate(
            [wso, xso[i * BPC].reshape(128, KC1, NHW)], axis=2)
        cc = np.concatenate(
            [wc, xc[i * BPC].reshape(128, KC1, NHW)], axis=2)
        in_maps.append({
            "x_so": xso[i * BPC:(i + 1) * BPC],
            "x_c": xc[i * BPC:(i + 1) * BPC],
            "comb_so": np.ascontiguousarray(cso.reshape(128, KC1 * RC)),
            "comb_c": np.ascontiguousarray(cc.reshape(128, KC1 * RC)),
            "w1": w1, "w2": w2, "bias": bias,
        })
    return _get_nc(thresh), in_maps


def kernel(**inputs):
    nc, in_maps = _prepare(inputs)
    res = run_bass_kernel_spmd(nc, in_maps, list(range(N_CORES)))
    out = np.concatenate([res.results[i]["out"].reshape(BPC, NN)
                          for i in range(N_CORES)], axis=0)
    return np.ascontiguousarray(out.reshape(B, NN, 1).astype(np.float32))
